# revision 11
# baseline (speedup 1.0000x reference)
"""Trainium2 Bass kernel for the DGCNN-style message-passing block.

Math (per batch b, data-parallel over 8 cores):
    proj = x @ Wp^T
    m[i] = max_k proj[knn[i,k]]           (edge maxpool: max_k(f_j - f_i) = m_i - proj_i)
    x1 = BN_l((m - proj) @ Wl^T);  x2 = BN_g(x @ Wg^T)
    h  = BN_1((x1+x2) @ W1^T + b1); a = sigmoid(BN_2(h @ W2^T + b2))
    out = BN_f(a*x1 + (1-a)*x2)

All BNs are inference-affine and fold into the weights host-side, and proj
composes into the local branch (x1 = m@Wl'^T - x@(Wl'Wp)^T + t_l), giving:
    f  = m@Wmf^T + x@Wxf^T + tf          (= x1+x2, feeds h)
    d' = m@Wmd^T + x@Wxd^T + td          (= s_f*(x1-x2))
    v  = x@Wxv^T + tv                    (= BN_f(x2))
    h  = f@W1'^T + t1;  a = sigmoid(h@W2'^T + t2)
    out = v + a*d'

On-chip layout is feature-major ([channel partitions, node free]); the host
supplies x pre-transposed and transposes the output back.  The KNN max-gather
runs as 16 SWDGE indirect DMAs (one per neighbor slot) from a bf16 node-major
copy of proj in DRAM, with CCE max-accumulate into two alternating SBUF tiles.
"""

import numpy as np
import ml_dtypes

import concourse.bass as bass
import concourse.mybir as mybir
import concourse.tile as tile
from concourse import bacc
from concourse.bass_utils import run_bass_kernel_spmd
from concourse.masks import make_identity

F32 = mybir.dt.float32
F32R = mybir.dt.float32r
BF16 = mybir.dt.bfloat16
I32 = mybir.dt.int32
I16 = mybir.dt.int16

B, N, K, C = 8, 4096, 16, 256
P = 128
NT = N // P          # 32 node tiles / stripes
CK = C // P          # 2 channel chunks
NQ = 4               # node quarters
QN = N // NQ         # 1024 nodes per quarter
EPS = 1e-5

AF = mybir.ActivationFunctionType
NEG_INF = float("-inf")


def build_bass(n_cores: int = 8):
    nc = bacc.Bacc(
        "TRN2",
        target_bir_lowering=False,
        debug=False,
        enable_asserts=False,
        num_devices=n_cores,
        num_swdge_queues=4,
    )

    xT = nc.dram_tensor("xT", [C, N], F32R, kind="ExternalInput").ap()
    knn_i = nc.dram_tensor("knn_i", [K, NQ, P, N // NQ // 16], I16, kind="ExternalInput").ap()
    wpT = nc.dram_tensor("wpT", [C, C], F32R, kind="ExternalInput").ap()
    wmf = nc.dram_tensor("wmf", [C, C], BF16, kind="ExternalInput").ap()
    wmd = nc.dram_tensor("wmd", [C, C], BF16, kind="ExternalInput").ap()
    wxf = nc.dram_tensor("wxf", [C, C], F32R, kind="ExternalInput").ap()
    wxd = nc.dram_tensor("wxd", [C, C], F32R, kind="ExternalInput").ap()
    wxv = nc.dram_tensor("wxv", [C, C], F32R, kind="ExternalInput").ap()
    w1 = nc.dram_tensor("w1", [C, C], BF16, kind="ExternalInput").ap()
    w2 = nc.dram_tensor("w2", [C, C], BF16, kind="ExternalInput").ap()
    bias = nc.dram_tensor("bias", [P, 10], F32, kind="ExternalInput").ap()
    outT = nc.dram_tensor("outT", [C, N], F32, kind="ExternalOutput").ap()

    with tile.TileContext(nc) as tc:
        kernel_body(tc, xT, knn_i, wpT, wmf, wmd, wxf, wxd, wxv, w1, w2, bias, outT)
    nc.compile()
    return nc


def kernel_body(tc, xT, knn_i, wpT, wmf, wmd, wxf, wxd, wxv, w1, w2, bias, outT):
    nc = tc.nc

    with (
        tc.tile_pool(name="const", bufs=1) as cpool,
        tc.tile_pool(name="projp", bufs=1) as projp,
        tc.tile_pool(name="gat", bufs=1) as gat,
        tc.tile_pool(name="mt", bufs=2) as mtp,
        tc.tile_pool(name="units", bufs=2) as up,
        tc.tile_pool(name="outp", bufs=2) as outp,
        tc.tile_pool(name="psA", bufs=2, space="PSUM") as psA,
        tc.tile_pool(name="psB", bufs=2, space="PSUM") as psB,
        tc.tile_pool(name="psNP", bufs=2, space="PSUM") as psNP,
        tc.tile_pool(name="dram", bufs=1, space="DRAM") as dpool,
    ):
        # ---- constants / inputs to SBUF ----
        xt = []
        for kc in range(CK):
            t = cpool.tile([P, N], F32R, name=f"xt{kc}")
            nc.sync.dma_start(t[:], xT[kc * P:(kc + 1) * P, :])
            xt.append(t)

        def wload(ap, dt, nm):
            ts = []
            for kc in range(CK):
                t = cpool.tile([P, C], dt, name=f"{nm}{kc}")
                nc.sync.dma_start(t[:], ap[kc * P:(kc + 1) * P, :])
                ts.append(t)
            return ts

        wpT_sb = wload(wpT, F32R, "wp")
        wmf_sb = wload(wmf, BF16, "wmf")
        wmd_sb = wload(wmd, BF16, "wmd")
        wxf_sb = wload(wxf, F32R, "wxf")
        wxd_sb = wload(wxd, F32R, "wxd")
        wxv_sb = wload(wxv, F32R, "wxv")
        w1_sb = wload(w1, BF16, "w1")
        w2_sb = wload(w2, BF16, "w2")

        bias_sb = cpool.tile([P, 10], F32)
        nc.sync.dma_start(bias_sb[:], bias[:])

        kidx = {}
        for k in range(K):
            for q in range(NQ):
                t = cpool.tile([P, QN // 16], I16, name=f"kidx{k}_{q}")
                nc.sync.dma_start(t[:], knn_i[k, q])
                kidx[(q, k)] = t

        ident = cpool.tile([P, P], BF16)
        make_identity(nc, ident[:])

        # ---- phase 1: proj, node-major bf16 -> DRAM ----
        # proj_dram row r = p*NT + s holds node n = s*P + p (p-major permutation,
        # host permutes the knn indices to match). SBUF->DRAM write is contiguous.
        proj_sb = projp.tile([P, NT, C], BF16)
        for t in range(NT):
            ps = psNP.tile([P, C], F32, name="ps_np", tag="ps_np")
            nc.tensor.matmul(ps[:], lhsT=xt[0][:, t * P:(t + 1) * P],
                             rhs=wpT_sb[0][:], start=True, stop=False)
            nc.tensor.matmul(ps[:], lhsT=xt[1][:, t * P:(t + 1) * P],
                             rhs=wpT_sb[1][:], start=False, stop=True)
            nc.scalar.activation(proj_sb[:, t, :], ps[:], AF.Copy)

        proj_dram = dpool.tile([N, C], BF16)
        nc.sync.dma_start(proj_dram[:].rearrange("(p s) c -> p (s c)", p=P), proj_sb[:])

        # ---- phases 2-4: per node-quarter pipeline ----
        # gather (16 x 1024-idx dma_gather over 4 SWDGE queues) + DVE max chain
        QT = QN // P  # stripes per quarter (8)
        for q in range(NQ):
            gk = [None] * K
            for k in range(K):
                g = gat.tile([P, QT, C], BF16, name=f"g_{q}_{k}", tag="gk", bufs=6)
                nc.gpsimd.dma_gather(
                    out_ap=g[:],
                    in_ap=proj_dram[:],
                    idxs_ap=kidx[(q, k)][:],
                    num_idxs=QN,
                    num_idxs_reg=QN,
                    elem_size=C,
                    queue_num=k % 4,
                )
                gk[k] = g
            acc = gat.tile([P, QT, C], BF16, name=f"acc{q}", tag="acc", bufs=2)
            nc.vector.tensor_tensor(out=acc[:], in0=gk[0][:], in1=gk[1][:],
                                    op=mybir.AluOpType.max)
            for k in range(2, K):
                nc.vector.tensor_tensor(out=acc[:], in0=acc[:], in1=gk[k][:],
                                        op=mybir.AluOpType.max)

            # m^T via PE transpose: [node, c] -> [c, node]
            mt = mtp.tile([P, CK, QN], BF16, name="mt", tag="mt")
            for kc in range(CK):
                pst = psB.tile([P, QN], BF16, name="pst", tag="pst")
                for j in range(QT):
                    nc.tensor.transpose(pst[:, j * P:(j + 1) * P],
                                        acc[:, j, kc * P:(kc + 1) * P], ident[:])
                nc.vector.tensor_copy(mt[:, kc, :], pst[:])

            f_sb = up.tile([P, CK, QN], BF16, name="f_sb", tag="f")
            d_sb = up.tile([P, CK, QN], F32, name="d_sb", tag="d")
            v_sb = up.tile([P, CK, QN], F32, name="v_sb", tag="v")
            h_sb = up.tile([P, CK, QN], BF16, name="h_sb", tag="h")
            a_sb = up.tile([P, CK, QN], F32, name="a_sb", tag="a")

            def mx_pass(out_sb, wm_sb, wx_sb, bcol, mc, func=AF.Identity):
                """psum = [m-part] + [x-part]; evac with per-channel bias."""
                ps = psA.tile([P, QN], F32, name="ps_fp", tag="ps_fp")
                first = True
                if wm_sb is not None:
                    for kc in range(CK):
                        for nh in range(QN // 512):
                            sl = slice(nh * 512, (nh + 1) * 512)
                            nc.tensor.matmul(
                                ps[:, sl], lhsT=wm_sb[kc][:, mc * P:(mc + 1) * P],
                                rhs=mt[:, kc, sl], start=first and kc == 0,
                                stop=False, skip_group_check=True)
                    first = False
                for kc in range(CK):
                    for nh in range(QN // 512):
                        sl = slice(nh * 512, (nh + 1) * 512)
                        last = (kc == CK - 1) and (nh == QN // 512 - 1)
                        nc.tensor.matmul(
                            ps[:, sl], lhsT=wx_sb[kc][:, mc * P:(mc + 1) * P],
                            rhs=xt[kc][:, q * QN:(q + 1) * QN][:, sl],
                            start=first and kc == 0, stop=last,
                            skip_group_check=True)
                nc.scalar.activation(out_sb[:, mc, :], ps[:], func,
                                     bias=bias_sb[:, bcol + mc:bcol + mc + 1],
                                     scale=1.0)

            def hx_pass(out_sb, w_sb, in_sb, bcol, mc, func=AF.Identity):
                """h/a-style pass: rhs comes from an on-chip unit tile."""
                ps = psA.tile([P, QN], F32, name="ps_fp", tag="ps_fp")
                for kc in range(CK):
                    for nh in range(QN // 512):
                        sl = slice(nh * 512, (nh + 1) * 512)
                        nc.tensor.matmul(
                            ps[:, sl], lhsT=w_sb[kc][:, mc * P:(mc + 1) * P],
                            rhs=in_sb[:, kc, sl],
                            start=kc == 0, stop=kc == CK - 1,
                            skip_group_check=True)
                nc.scalar.activation(out_sb[:, mc, :], ps[:], func,
                                     bias=bias_sb[:, bcol + mc:bcol + mc + 1],
                                     scale=1.0)

            for mc in range(CK):
                mx_pass(f_sb, wmf_sb, wxf_sb, 0, mc)
                mx_pass(d_sb, wmd_sb, wxd_sb, 2, mc)
                mx_pass(v_sb, None, wxv_sb, 4, mc)
            for mc in range(CK):
                hx_pass(h_sb, w1_sb, f_sb, 6, mc)
            for mc in range(CK):
                hx_pass(a_sb, w2_sb, h_sb, 8, mc, func=AF.Sigmoid)

            # out = v + a*d'
            for mc in range(CK):
                ot = outp.tile([P, QN], F32, name="ot", tag="ot")
                nc.vector.tensor_tensor(out=ot[:], in0=a_sb[:, mc, :],
                                        in1=d_sb[:, mc, :],
                                        op=mybir.AluOpType.mult)
                nc.vector.tensor_tensor(out=ot[:], in0=ot[:], in1=v_sb[:, mc, :],
                                        op=mybir.AluOpType.add)
                nc.sync.dma_start(outT[mc * P:(mc + 1) * P, q * QN:(q + 1) * QN], ot[:])


# ---------------- host side ----------------

def _fold(proj_W, local_W, glob_W, aff_W1, aff_b1, aff_W2, aff_b2,
          bn_local, bn_glob, bn_aff1, bn_aff2, bn_final):
    f32 = np.float32

    def bn_st(p):
        p = np.asarray(p, f32)
        g, b, m, v = p
        s = g / np.sqrt(v + EPS)
        return s.astype(f32), (b - m * s).astype(f32)

    Wp = np.asarray(proj_W, f32)
    s_l, t_l = bn_st(bn_local)
    s_g, t_g = bn_st(bn_glob)
    s_1, t_1 = bn_st(bn_aff1)
    s_2, t_2 = bn_st(bn_aff2)
    s_f, t_f = bn_st(bn_final)

    Wlp = s_l[:, None] * np.asarray(local_W, f32)
    Wgp = s_g[:, None] * np.asarray(glob_W, f32)
    Wlproj = (Wlp @ Wp).astype(f32)

    w = {}
    w["wpT"] = np.ascontiguousarray(Wp.T)
    w["wmf"] = np.ascontiguousarray(Wlp.T).astype(ml_dtypes.bfloat16)
    w["wmd"] = np.ascontiguousarray((s_f[:, None] * Wlp).T).astype(ml_dtypes.bfloat16)
    w["wxf"] = np.ascontiguousarray((Wgp - Wlproj).T)
    w["wxd"] = np.ascontiguousarray((-s_f[:, None] * (Wlproj + Wgp)).T)
    w["wxv"] = np.ascontiguousarray((s_f[:, None] * Wgp).T)
    w["w1"] = np.ascontiguousarray((s_1[:, None] * np.asarray(aff_W1, f32)).T).astype(ml_dtypes.bfloat16)
    w["w2"] = np.ascontiguousarray((s_2[:, None] * np.asarray(aff_W2, f32)).T).astype(ml_dtypes.bfloat16)

    tf = t_l + t_g
    td = s_f * (t_l - t_g)
    tv = s_f * t_g + t_f
    t1 = s_1 * np.asarray(aff_b1, f32) + t_1
    t2 = s_2 * np.asarray(aff_b2, f32) + t_2
    # bias[p, 2*j + mc] = coeff_j[mc*128 + p]
    bias = np.zeros((P, 10), f32)
    for j, tt in enumerate((tf, td, tv, t1, t2)):
        for mc in range(CK):
            bias[:, 2 * j + mc] = tt[mc * P:(mc + 1) * P]
    w["bias"] = bias
    return w


_NC_CACHE = {}


def _get_nc():
    if "nc" not in _NC_CACHE:
        _NC_CACHE["nc"] = build_bass(B)
    return _NC_CACHE["nc"]


def kernel(**inputs) -> np.ndarray:
    x = np.ascontiguousarray(np.asarray(inputs["x"], np.float32))      # [B,N,C]
    knn = np.asarray(inputs["knn"]).astype(np.int64)                   # [B,N,K]
    w = _fold(
        inputs["proj_W"], inputs["local_W"], inputs["glob_W"],
        inputs["aff_W1"], inputs["aff_b1"], inputs["aff_W2"], inputs["aff_b2"],
        inputs["bn_local"], inputs["bn_glob"], inputs["bn_aff1"],
        inputs["bn_aff2"], inputs["bn_final"],
    )

    # proj_dram row permutation: node n lives at row (n%128)*32 + n//128
    r = ((knn % P) * NT + knn // P).astype(np.int16)                   # [B,N,K]
    # dma_gather wrapped layout: per (k, half), flat i -> [i%16, i//16],
    # replicated 8x across partition groups -> [B, K, 2, 128, 128]
    rr = r.reshape(B, NQ, N // NQ, K)                  # [B, 4, 1024, K]
    wrapped = rr.reshape(B, NQ, N // NQ // 16, 16, K).transpose(0, 4, 1, 3, 2)
    ridx = np.tile(wrapped, (1, 1, 1, 8, 1)).astype(np.int16)  # [B, K, 4, 128, 64]

    nc = _get_nc()
    in_maps = []
    for b in range(B):
        m = {"xT": np.ascontiguousarray(x[b].T),
             "knn_i": np.ascontiguousarray(ridx[b])}
        for k2, v in w.items():
            m[k2] = v
        in_maps.append(m)

    res = run_bass_kernel_spmd(nc, in_maps, core_ids=list(range(B)))
    out = np.stack([res.results[b]["outT"].T for b in range(B)])
    return out.astype(np.float32)


if __name__ == "__main__":
    nc = build_bass(1)
    print("built OK;", len(nc.m.functions[0].instructions), "instructions")


# revision 17
# speedup vs baseline: 5.6173x; 5.6173x over previous
"""Trainium2 Bass kernel for the DGCNN-style message-passing block.

Math (per batch b, data-parallel over 8 cores):
    proj = x @ Wp^T
    m[i] = max_k proj[knn[i,k]]           (edge maxpool: max_k(f_j - f_i) = m_i - proj_i)
    x1 = BN_l((m - proj) @ Wl^T);  x2 = BN_g(x @ Wg^T)
    h  = BN_1((x1+x2) @ W1^T + b1); a = sigmoid(BN_2(h @ W2^T + b2))
    out = BN_f(a*x1 + (1-a)*x2)

All BNs are inference-affine and fold into the weights host-side, and proj
composes into the local branch (x1 = m@Wl'^T - x@(Wl'Wp)^T + t_l), giving:
    f  = m@Wmf^T + x@Wxf^T + tf          (= x1+x2, feeds h)
    d' = m@Wmd^T + x@Wxd^T + td          (= s_f*(x1-x2))
    v  = x@Wxv^T + tv                    (= BN_f(x2))
    h  = f@W1'^T + t1;  a = sigmoid(h@W2'^T + t2)
    out = v + a*d'

On-chip layout is feature-major ([channel partitions, node free]); the host
supplies x pre-transposed and transposes the output back.  The KNN max-gather
runs as 16 SWDGE indirect DMAs (one per neighbor slot) from a bf16 node-major
copy of proj in DRAM, with CCE max-accumulate into two alternating SBUF tiles.
"""

import numpy as np
import ml_dtypes

import concourse.bass as bass
import concourse.mybir as mybir
import concourse.tile as tile
from concourse import bacc
from concourse.bass_utils import run_bass_kernel_spmd
from concourse.masks import make_identity

F32 = mybir.dt.float32
F32R = mybir.dt.float32r
BF16 = mybir.dt.bfloat16
I32 = mybir.dt.int32
I16 = mybir.dt.int16

B, N, K, C = 8, 4096, 16, 256
P = 128
NT = N // P          # 32 node tiles / stripes
CK = C // P          # 2 channel chunks
NQ = 4               # node quarters
QN = N // NQ         # 1024 nodes per quarter
EPS = 1e-5

AF = mybir.ActivationFunctionType
NEG_INF = float("-inf")


def build_bass(n_cores: int = 8, reps: int = 1):
    nc = bacc.Bacc(
        "TRN2",
        target_bir_lowering=False,
        debug=False,
        enable_asserts=False,
        num_devices=n_cores,
        num_swdge_queues=4,
    )

    xT = nc.dram_tensor("xT", [C, N], F32R, kind="ExternalInput").ap()
    knn_i = nc.dram_tensor("knn_i", [K * NQ, P, N // NQ // 16], I16, kind="ExternalInput").ap()
    # packed weights: [128, (w, kc, 256)]; f32r order: wpT,wxf,wxd,wxv; bf16: wmf,wmd,w1,w2
    wf = nc.dram_tensor("wf", [P, 4 * CK * C], F32R, kind="ExternalInput").ap()
    wb = nc.dram_tensor("wb", [P, 4 * CK * C], BF16, kind="ExternalInput").ap()
    bias = nc.dram_tensor("bias", [P, 10], F32, kind="ExternalInput").ap()
    outT = nc.dram_tensor("outT", [C, N], F32, kind="ExternalOutput").ap()

    with tile.TileContext(nc) as tc:
        for _ in range(reps):
            kernel_body(tc, xT, knn_i, wf, wb, bias, outT)
    nc.compile()
    return nc


def kernel_body(tc, xT, knn_i, wf, wb, bias, outT):
    nc = tc.nc

    with (
        tc.tile_pool(name="const", bufs=1) as cpool,
        tc.tile_pool(name="projp", bufs=1) as projp,
        tc.tile_pool(name="gat", bufs=1) as gat,
        tc.tile_pool(name="mt", bufs=2) as mtp,
        tc.tile_pool(name="units", bufs=2) as up,
        tc.tile_pool(name="outp", bufs=2) as outp,
        tc.tile_pool(name="psA", bufs=3, space="PSUM") as psA,
        tc.tile_pool(name="psB", bufs=1, space="PSUM") as psB,
        tc.tile_pool(name="psNP", bufs=1, space="PSUM") as psNP,
        tc.tile_pool(name="dram", bufs=1, space="DRAM") as dpool,
    ):
        # ---- constants / inputs to SBUF ----
        # xt[kc][cc]: [128, QN] chunk of x^T, loaded per-quarter for overlap
        xt = []
        for kc in range(CK):
            row = []
            for cc in range(NQ):
                t = cpool.tile([P, QN], F32R, name=f"xt{kc}_{cc}")
                nc.sync.dma_start(t[:], xT[kc * P:(kc + 1) * P, cc * QN:(cc + 1) * QN])
                row.append(t)
            xt.append(row)

        wf_sb = cpool.tile([P, 4 * CK * C], F32R)
        nc.sync.dma_start(wf_sb[:], wf[:])
        wb_sb = cpool.tile([P, 4 * CK * C], BF16)
        nc.sync.dma_start(wb_sb[:], wb[:])

        def wslice(sb, w_i):
            return [sb[:, (w_i * CK + kc) * C:(w_i * CK + kc + 1) * C] for kc in range(CK)]

        wpT_sb = wslice(wf_sb, 0)
        wxf_sb = wslice(wf_sb, 1)
        wxd_sb = wslice(wf_sb, 2)
        wxv_sb = wslice(wf_sb, 3)
        wmf_sb = wslice(wb_sb, 0)
        wmd_sb = wslice(wb_sb, 1)
        w1_sb = wslice(wb_sb, 2)
        w2_sb = wslice(wb_sb, 3)

        bias_sb = cpool.tile([P, 10], F32)
        nc.sync.dma_start(bias_sb[:], bias[:])

        COLS_Q = QN // 16
        kidx_all = cpool.tile([P, K * NQ * COLS_Q], I16)
        nc.sync.dma_start(
            kidx_all[:].rearrange("p (kq c) -> p kq c", c=COLS_Q),
            knn_i[:].rearrange("kq p c -> p kq c"))
        kidx = {}
        for k in range(K):
            for q in range(NQ):
                kq = k * NQ + q
                kidx[(q, k)] = kidx_all[:, kq * COLS_Q:(kq + 1) * COLS_Q]

        ident = cpool.tile([P, P], BF16)
        make_identity(nc, ident[:])

        # ---- phase 1: proj, node-major bf16 -> DRAM ----
        # proj_dram row r = p*NT + s holds node n = s*P + p (p-major permutation,
        # host permutes the knn indices to match). SBUF->DRAM write is contiguous.
        proj_sb = projp.tile([P, NT, C], BF16)
        for t in range(NT):
            ps = psNP.tile([P, C], F32, name="ps_np", tag="ps_np")
            cc, tl = t // (QN // P), t % (QN // P)
            nc.tensor.matmul(ps[:], lhsT=xt[0][cc][:, tl * P:(tl + 1) * P],
                             rhs=wpT_sb[0], start=True, stop=False)
            nc.tensor.matmul(ps[:], lhsT=xt[1][cc][:, tl * P:(tl + 1) * P],
                             rhs=wpT_sb[1], start=False, stop=True)
            nc.scalar.activation(proj_sb[:, t, :], ps[:], AF.Copy)

        proj_dram = dpool.tile([N, C], BF16)
        nc.sync.dma_start(proj_dram[:].rearrange("(p s) c -> p (s c)", p=P), proj_sb[:])

        # ---- phases 2-4: per node-quarter pipeline ----
        # gather (16 x 1024-idx dma_gather over 4 SWDGE queues) + DVE max chain
        QT = QN // P  # stripes per quarter (8)
        for q in range(NQ):
            gk = [None] * K
            for k in range(K):
                g = gat.tile([P, QT, C], BF16, name=f"g_{q}_{k}", tag="gk", bufs=8)
                nc.gpsimd.dma_gather(
                    out_ap=g[:],
                    in_ap=proj_dram[:],
                    idxs_ap=kidx[(q, k)],
                    num_idxs=QN,
                    num_idxs_reg=QN,
                    elem_size=C,
                    queue_num=k % 4,
                )
                gk[k] = g
            acc = gat.tile([P, QT, C], BF16, name=f"acc{q}", tag="acc", bufs=2)
            nc.vector.tensor_tensor(out=acc[:], in0=gk[0][:], in1=gk[1][:],
                                    op=mybir.AluOpType.max)
            for k in range(2, K):
                nc.vector.tensor_tensor(out=acc[:], in0=acc[:], in1=gk[k][:],
                                        op=mybir.AluOpType.max)

            # m^T via PE transpose: [node, c] -> [c, node]
            mt = mtp.tile([P, CK, QN], BF16, name="mt", tag="mt")
            for kc in range(CK):
                pst = psB.tile([P, QN], BF16, name="pst", tag="pst")
                for j in range(QT):
                    nc.tensor.transpose(pst[:, j * P:(j + 1) * P],
                                        acc[:, j, kc * P:(kc + 1) * P], ident[:])
                nc.vector.tensor_copy(mt[:, kc, :], pst[:])

            f_sb = up.tile([P, CK, QN], BF16, name="f_sb", tag="f")
            d_sb = up.tile([P, CK, QN], F32, name="d_sb", tag="d")
            v_sb = up.tile([P, CK, QN], F32, name="v_sb", tag="v")
            h_sb = up.tile([P, CK, QN], BF16, name="h_sb", tag="h")
            a_sb = up.tile([P, CK, QN], F32, name="a_sb", tag="a")

            def mx_pass(out_sb, wm_sb, wx_sb, bcol, mc, func=AF.Identity):
                """psum = [m-part] + [x-part]; evac with per-channel bias."""
                ps = psA.tile([P, QN], F32, name="ps_fp", tag="ps_fp")
                first = True
                if wm_sb is not None:
                    for kc in range(CK):
                        for nh in range(QN // 512):
                            sl = slice(nh * 512, (nh + 1) * 512)
                            nc.tensor.matmul(
                                ps[:, sl], lhsT=wm_sb[kc][:, mc * P:(mc + 1) * P],
                                rhs=mt[:, kc, sl], start=first and kc == 0,
                                stop=False, skip_group_check=True)
                    first = False
                for kc in range(CK):
                    for nh in range(QN // 512):
                        sl = slice(nh * 512, (nh + 1) * 512)
                        last = (kc == CK - 1) and (nh == QN // 512 - 1)
                        nc.tensor.matmul(
                            ps[:, sl], lhsT=wx_sb[kc][:, mc * P:(mc + 1) * P],
                            rhs=xt[kc][q][:, sl],
                            start=first and kc == 0, stop=last,
                            skip_group_check=True)
                nc.scalar.activation(out_sb[:, mc, :], ps[:], func,
                                     bias=bias_sb[:, bcol + mc:bcol + mc + 1],
                                     scale=1.0)

            def hx_pass(out_sb, w_sb, in_sb, bcol, mc, func=AF.Identity):
                """h/a-style pass: rhs comes from an on-chip unit tile."""
                ps = psA.tile([P, QN], F32, name="ps_fp", tag="ps_fp")
                for kc in range(CK):
                    for nh in range(QN // 512):
                        sl = slice(nh * 512, (nh + 1) * 512)
                        nc.tensor.matmul(
                            ps[:, sl], lhsT=w_sb[kc][:, mc * P:(mc + 1) * P],
                            rhs=in_sb[:, kc, sl],
                            start=kc == 0, stop=kc == CK - 1,
                            skip_group_check=True)
                nc.scalar.activation(out_sb[:, mc, :], ps[:], func,
                                     bias=bias_sb[:, bcol + mc:bcol + mc + 1],
                                     scale=1.0)

            for mc in range(CK):
                mx_pass(f_sb, wmf_sb, wxf_sb, 0, mc)
                mx_pass(d_sb, wmd_sb, wxd_sb, 2, mc)
                mx_pass(v_sb, None, wxv_sb, 4, mc)
            for mc in range(CK):
                hx_pass(h_sb, w1_sb, f_sb, 6, mc)
            for mc in range(CK):
                hx_pass(a_sb, w2_sb, h_sb, 8, mc, func=AF.Sigmoid)

            # out = v + a*d'
            for mc in range(CK):
                ot = outp.tile([P, QN], F32, name="ot", tag="ot")
                nc.vector.tensor_tensor(out=ot[:], in0=a_sb[:, mc, :],
                                        in1=d_sb[:, mc, :],
                                        op=mybir.AluOpType.mult)
                nc.vector.tensor_tensor(out=ot[:], in0=ot[:], in1=v_sb[:, mc, :],
                                        op=mybir.AluOpType.add)
                nc.sync.dma_start(outT[mc * P:(mc + 1) * P, q * QN:(q + 1) * QN], ot[:])


# ---------------- host side ----------------

def _fold(proj_W, local_W, glob_W, aff_W1, aff_b1, aff_W2, aff_b2,
          bn_local, bn_glob, bn_aff1, bn_aff2, bn_final):
    f32 = np.float32

    def bn_st(p):
        p = np.asarray(p, f32)
        g, b, m, v = p
        s = g / np.sqrt(v + EPS)
        return s.astype(f32), (b - m * s).astype(f32)

    Wp = np.asarray(proj_W, f32)
    s_l, t_l = bn_st(bn_local)
    s_g, t_g = bn_st(bn_glob)
    s_1, t_1 = bn_st(bn_aff1)
    s_2, t_2 = bn_st(bn_aff2)
    s_f, t_f = bn_st(bn_final)

    Wlp = s_l[:, None] * np.asarray(local_W, f32)
    Wgp = s_g[:, None] * np.asarray(glob_W, f32)
    Wlproj = (Wlp @ Wp).astype(f32)

    def pack(ws, dt):
        # ws: list of [C, C] W^T arrays -> [128, n*CK*C]: block (w_i, kc) = W^T[kc*128:(kc+1)*128, :]
        P_, CK_ = 128, 2
        out = np.zeros((P_, len(ws) * CK_ * 256), dt)
        for w_i, m in enumerate(ws):
            for kc in range(CK_):
                out[:, (w_i * CK_ + kc) * 256:(w_i * CK_ + kc + 1) * 256] = m[kc * P_:(kc + 1) * P_, :].astype(dt)
        return out

    w = {}
    wpT = np.ascontiguousarray(Wp.T)
    wxf = np.ascontiguousarray((Wgp - Wlproj).T)
    wxd = np.ascontiguousarray((-s_f[:, None] * (Wlproj + Wgp)).T)
    wxv = np.ascontiguousarray((s_f[:, None] * Wgp).T)
    wmf = np.ascontiguousarray(Wlp.T)
    wmd = np.ascontiguousarray((s_f[:, None] * Wlp).T)
    w1 = np.ascontiguousarray((s_1[:, None] * np.asarray(aff_W1, f32)).T)
    w2 = np.ascontiguousarray((s_2[:, None] * np.asarray(aff_W2, f32)).T)
    w["wf"] = pack([wpT, wxf, wxd, wxv], np.float32)
    w["wb"] = pack([wmf, wmd, w1, w2], ml_dtypes.bfloat16)

    tf = t_l + t_g
    td = s_f * (t_l - t_g)
    tv = s_f * t_g + t_f
    t1 = s_1 * np.asarray(aff_b1, f32) + t_1
    t2 = s_2 * np.asarray(aff_b2, f32) + t_2
    # bias[p, 2*j + mc] = coeff_j[mc*128 + p]
    bias = np.zeros((P, 10), f32)
    for j, tt in enumerate((tf, td, tv, t1, t2)):
        for mc in range(CK):
            bias[:, 2 * j + mc] = tt[mc * P:(mc + 1) * P]
    w["bias"] = bias
    return w


_NC_CACHE = {}


def _get_nc():
    if "nc" not in _NC_CACHE:
        _NC_CACHE["nc"] = build_bass(B)
    return _NC_CACHE["nc"]


def kernel(**inputs) -> np.ndarray:
    x = np.ascontiguousarray(np.asarray(inputs["x"], np.float32))      # [B,N,C]
    knn = np.asarray(inputs["knn"]).astype(np.int64)                   # [B,N,K]
    w = _fold(
        inputs["proj_W"], inputs["local_W"], inputs["glob_W"],
        inputs["aff_W1"], inputs["aff_b1"], inputs["aff_W2"], inputs["aff_b2"],
        inputs["bn_local"], inputs["bn_glob"], inputs["bn_aff1"],
        inputs["bn_aff2"], inputs["bn_final"],
    )

    # proj_dram row permutation: node n lives at row (n%128)*32 + n//128
    r = ((knn % P) * NT + knn // P).astype(np.int16)                   # [B,N,K]
    # dma_gather wrapped layout: per (k, half), flat i -> [i%16, i//16],
    # replicated 8x across partition groups -> [B, K, 2, 128, 128]
    rr = r.reshape(B, NQ, N // NQ, K)                  # [B, 4, 1024, K]
    wrapped = rr.reshape(B, NQ, N // NQ // 16, 16, K).transpose(0, 4, 1, 3, 2)
    ridx = np.tile(wrapped, (1, 1, 1, 8, 1)).astype(np.int16).reshape(B, K * NQ, 128, -1)  # [B, K*4, 128, 64]

    nc = _get_nc()
    in_maps = []
    for b in range(B):
        m = {"xT": np.ascontiguousarray(x[b].T),
             "knn_i": np.ascontiguousarray(ridx[b])}
        for k2, v in w.items():
            m[k2] = v
        in_maps.append(m)

    res = run_bass_kernel_spmd(nc, in_maps, core_ids=list(range(B)))
    out = np.stack([res.results[b]["outT"].T for b in range(B)])
    return out.astype(np.float32)


if __name__ == "__main__":
    nc = build_bass(1)
    print("built OK;", len(nc.m.functions[0].instructions), "instructions")
